# revision 15
# baseline (speedup 1.0000x reference)
"""Bass/Trainium2 SPMD kernel for nn_Decoder_9311489098140.

Pure data parallel over the batch axis: each of the 8 NeuronCores gets
B/8 = 8192 rows of z plus (host-preprocessed) replicated weights.

All matmuls are bf16 (trn2 walrus cannot attach >1 sem wait to
self-loading fp32 matmuls, and fp32 runs at 1/4 rate anyway).  The
size-prediction path needs ~fp32 precision (argmax feeds the mask), so
it uses hi/lo bf16 splits: a@b ~= ah@bh + al@bh + ah@bl with fp32 PSUM
accumulation (~2^-17 relative error; verified 1 argmax flip in 65536
rows vs the f32 reference, with no measurable output error).

Per-core pipeline (super-tile = 512 rows, chunk = 128 rows):
  - z arrives pre-transposed and pre-split (zT_hi/zT_lo bf16) so no
    on-device transposes are needed; zT_hi doubles as the decoder input.
  - decoder path: position-modulated weights W1all[:, p*64+j] =
    pe[p,i]*dec_w1[i,j] built on host; layer 1 computes X^T for 2
    positions per matmul (full 128 partitions), gelu adds dec_b1 as a
    per-partition ACT bias, layer 2 multiplies by blockdiag(dec_w2) to
    produce row-major x.  The validity mask is applied for free by
    copy_predicated into a zeroed tile during the PSUM->SBUF copy.
  - argmax via DVE max/max_index; mask = iota < argmax.
  - batch output is input-independent (broadcast arange) -> host numpy.
"""

import os
import sys

for _p in (
    "/opt/trn_rl_repo",
    "/root/.axon_site",
    "/root/.axon_site/_ro/trn_rl_repo",
    "/root/.axon_site/_ro/pypackages",
):
    if os.path.isdir(_p) and _p not in sys.path:
        sys.path.append(_p)

import numpy as np
import ml_dtypes

from concourse import bass, bacc, mybir, tile
from concourse.bass_utils import run_bass_kernel_spmd

N_CORES = 8
B, H, D, MAXN = 65536, 64, 64, 16
HID = 64                      # decoder hidden width (dec_w1: [H, HID])
SPH = 40                      # size-pred hidden width
BSH = B // N_CORES            # 8192 rows per core
SUP = 512                     # rows per super-tile
CH = 128                      # rows per chunk
NSUP = BSH // SUP             # 16
NCH = SUP // CH               # 4
NPAIR = MAXN // 2             # 8 position-pairs

F32 = mybir.dt.float32
BF16 = mybir.dt.bfloat16
I32 = mybir.dt.int32
U8 = mybir.dt.uint8
U32 = mybir.dt.uint32
AF = mybir.ActivationFunctionType

LAST_EXEC_NS = None
LAST_RESULTS = None


def _gelu_np(x):
    x = np.asarray(x, np.float32)
    c = np.float32(np.sqrt(2.0 / np.pi))
    return (0.5 * x * (1.0 + np.tanh(c * (x + np.float32(0.044715) * x * x * x)))).astype(np.float32)


def _bf16(x):
    return np.asarray(x, dtype=ml_dtypes.bfloat16)


def _split(x):
    """x (f32) ~= hi + lo with both parts bf16."""
    x = np.asarray(x, np.float32)
    hi = _bf16(x)
    lo = _bf16(x - hi.astype(np.float32))
    return hi, lo


def build_nc(has_sp_b2: bool, has_dec_b2: bool):
    nc = bacc.Bacc()

    z_th = nc.declare_dram_parameter("zTh", [H, BSH], BF16, isOutput=False)
    z_tl = nc.declare_dram_parameter("zTl", [H, BSH], BF16, isOutput=False)
    w1all = nc.declare_dram_parameter("w1all", [H, MAXN * HID], BF16, isOutput=False)
    w2blk = nc.declare_dram_parameter("w2blk", [2 * HID, 2 * D], BF16, isOutput=False)
    b1pair = nc.declare_dram_parameter("b1pair", [2 * HID, 1], F32, isOutput=False)
    spw1h = nc.declare_dram_parameter("spw1h", [H, SPH], BF16, isOutput=False)
    spw1l = nc.declare_dram_parameter("spw1l", [H, SPH], BF16, isOutput=False)
    spb1 = nc.declare_dram_parameter("spb1", [SPH, 1], F32, isOutput=False)
    spw2h = nc.declare_dram_parameter("spw2h", [SPH, MAXN], BF16, isOutput=False)
    spw2l = nc.declare_dram_parameter("spw2l", [SPH, MAXN], BF16, isOutput=False)
    iotain = nc.declare_dram_parameter("iotain", [CH, MAXN], I32, isOutput=False)
    onesin = nc.declare_dram_parameter("onesin", [1, CH], BF16, isOutput=False)
    spb2h = nc.declare_dram_parameter("spb2h", [1, MAXN], BF16, isOutput=False)
    spb2l = nc.declare_dram_parameter("spb2l", [1, MAXN], BF16, isOutput=False)
    b2rep = nc.declare_dram_parameter("b2rep", [1, 8 * D], BF16, isOutput=False)

    out_x = nc.declare_dram_parameter("out_x", [BSH, MAXN * D], F32, isOutput=True)
    out_np = nc.declare_dram_parameter("out_np", [BSH, MAXN], F32, isOutput=True)
    out_mask = nc.declare_dram_parameter("out_mask", [BSH, MAXN], U8, isOutput=True)

    with tile.TileContext(nc) as tc:
        with (
            tc.tile_pool(name="const", bufs=1) as constp,
            tc.tile_pool(name="h1", bufs=2) as h1p,
            tc.tile_pool(name="g", bufs=2) as gp,
            tc.tile_pool(name="xsb", bufs=4) as xsbp,
            tc.tile_pool(name="npm", bufs=3) as npmp,
            tc.tile_pool(name="small", bufs=4) as smallp,
            tc.tile_pool(name="ps_h1", bufs=1, space="PSUM") as ps_h1,
            tc.tile_pool(name="ps_np", bufs=2, space="PSUM") as ps_np,
            tc.tile_pool(name="ps_big", bufs=2, space="PSUM") as ps_big,
        ):
            w1all_sb = constp.tile([H, MAXN * HID], BF16, tag="w1all")
            nc.sync.dma_start(w1all_sb[:], w1all[:])
            w2blk_sb = constp.tile([2 * HID, 2 * D], BF16, tag="w2blk")
            nc.sync.dma_start(w2blk_sb[:], w2blk[:])
            b1pair_sb = constp.tile([2 * HID, 1], F32, tag="b1pair")
            nc.sync.dma_start(b1pair_sb[:], b1pair[:])
            spw1h_sb = constp.tile([H, SPH], BF16, tag="spw1h")
            nc.sync.dma_start(spw1h_sb[:], spw1h[:])
            spw1l_sb = constp.tile([H, SPH], BF16, tag="spw1l")
            nc.sync.dma_start(spw1l_sb[:], spw1l[:])
            spb1_sb = constp.tile([SPH, 1], F32, tag="spb1")
            nc.sync.dma_start(spb1_sb[:], spb1[:])
            spw2h_sb = constp.tile([SPH, MAXN], BF16, tag="spw2h")
            nc.sync.dma_start(spw2h_sb[:], spw2h[:])
            spw2l_sb = constp.tile([SPH, MAXN], BF16, tag="spw2l")
            nc.sync.dma_start(spw2l_sb[:], spw2l[:])
            iota16 = constp.tile([CH, MAXN], I32, tag="iota16")
            nc.sync.dma_start(iota16[:], iotain[:])
            if has_sp_b2 or has_dec_b2:
                ones_b = constp.tile([1, CH], BF16, tag="ones_b")
                nc.sync.dma_start(ones_b[:], onesin[:])
            if has_sp_b2:
                spb2h_sb = constp.tile([1, MAXN], BF16, tag="spb2h")
                nc.sync.dma_start(spb2h_sb[:], spb2h[:])
                spb2l_sb = constp.tile([1, MAXN], BF16, tag="spb2l")
                nc.sync.dma_start(spb2l_sb[:], spb2l[:])
            if has_dec_b2:
                b2rep_sb = constp.tile([1, 8 * D], BF16, tag="b2rep")
                nc.sync.dma_start(b2rep_sb[:], b2rep[:])

            zth_sb = constp.tile([H, BSH], BF16, tag="zth_full")
            nc.sync.dma_start(zth_sb[:], z_th[:])
            ztl_sb = constp.tile([H, BSH], BF16, tag="ztl_full")
            nc.sync.dma_start(ztl_sb[:], z_tl[:])

            for s in range(NSUP):
                r0 = s * SUP
                zth = zth_sb[:, r0:r0 + SUP]
                ztl = ztl_sb[:, r0:r0 + SUP]

                # ---- size-prediction path (split-bf16 ~ fp32) ----
                h1ps = ps_h1.tile([SPH, SUP], F32, tag="h1ps")
                nc.tensor.matmul(h1ps[:], spw1h_sb[:], zth, start=True, stop=False)
                nc.tensor.matmul(h1ps[:], spw1h_sb[:], ztl, start=False, stop=False)
                nc.tensor.matmul(h1ps[:], spw1l_sb[:], zth, start=False, stop=True)
                h1 = h1p.tile([SPH, SUP], F32, tag="h1")
                nc.scalar.activation(h1[:], h1ps[:], AF.Gelu_apprx_tanh,
                                     bias=spb1_sb[:, 0:1])
                h1h = h1p.tile([SPH, SUP], BF16, tag="h1h")
                nc.vector.tensor_copy(h1h[:], h1[:])
                h1l = h1p.tile([SPH, SUP], BF16, tag="h1l")
                nc.vector.tensor_tensor(h1l[:], h1[:], h1h[:],
                                        mybir.AluOpType.subtract)

                npred_ps = ps_np.tile([CH, NCH * MAXN], F32, tag="npred_ps")
                for r in range(NCH):
                    c0 = r * MAXN
                    cc = slice(r * CH, (r + 1) * CH)
                    nc.tensor.matmul(npred_ps[:, c0:c0 + MAXN],
                                     h1h[:, cc], spw2h_sb[:],
                                     start=True, stop=False)
                    nc.tensor.matmul(npred_ps[:, c0:c0 + MAXN],
                                     h1l[:, cc], spw2h_sb[:],
                                     start=False, stop=False)
                    nc.tensor.matmul(npred_ps[:, c0:c0 + MAXN],
                                     h1h[:, cc], spw2l_sb[:],
                                     start=False, stop=not has_sp_b2)
                    if has_sp_b2:
                        nc.tensor.matmul(npred_ps[:, c0:c0 + MAXN],
                                         ones_b[:], spb2h_sb[:],
                                         start=False, stop=False)
                        nc.tensor.matmul(npred_ps[:, c0:c0 + MAXN],
                                         ones_b[:], spb2l_sb[:],
                                         start=False, stop=True)
                npred_sb = npmp.tile([CH, NCH * MAXN], F32, tag="npred_sb")
                nc.vector.tensor_copy(npred_sb[:], npred_ps[:])

                mask_sb = npmp.tile([CH, NCH * MAXN], U8, tag="mask_sb")
                for r in range(NCH):
                    c0 = r * MAXN
                    mx = smallp.tile([CH, 8], F32, tag="mx")
                    nc.vector.max(mx[:], npred_sb[:, c0:c0 + MAXN])
                    mi = smallp.tile([CH, 8], U32, tag="mi")
                    nc.vector.max_index(mi[:], mx[:], npred_sb[:, c0:c0 + MAXN])
                    mi_f = smallp.tile([CH, 1], F32, tag="mi_f")
                    nc.vector.tensor_copy(mi_f[:], mi[:, 0:1])
                    nc.vector.tensor_scalar(mask_sb[:, c0:c0 + MAXN], iota16[:],
                                            mi_f[:, 0:1], None,
                                            mybir.AluOpType.is_lt)

                np_dst = out_np[r0:r0 + SUP, :].rearrange("(r p) j -> p r j", p=CH)
                nc.sync.dma_start(np_dst,
                                  npred_sb[:].rearrange("p (r j) -> p r j", r=NCH))
                mk_dst = out_mask[r0:r0 + SUP, :].rearrange("(r p) j -> p r j", p=CH)
                nc.sync.dma_start(mk_dst,
                                  mask_sb[:].rearrange("p (r j) -> p r j", r=NCH))

                # ---- decoder path (bf16) ----
                for r in range(NCH):
                    cc = slice(r * CH, (r + 1) * CH)
                    xps = ps_big.tile([CH, MAXN * HID], F32, tag="ps_big")
                    for q in range(NPAIR):
                        nc.tensor.matmul(xps[:, q * 128:(q + 1) * 128],
                                         w1all_sb[:, q * 128:(q + 1) * 128],
                                         zth_sb[:, r0 + r * CH:r0 + (r + 1) * CH],
                                         start=True, stop=True)
                    g = gp.tile([CH, MAXN * HID], BF16, tag="g")
                    nc.scalar.activation(g[:], xps[:], AF.Gelu_apprx_tanh,
                                         bias=b1pair_sb[:, 0:1])

                    ops = ps_big.tile([CH, MAXN * D], F32, tag="ps_big")
                    for q in range(NPAIR):
                        nc.tensor.matmul(ops[:, q * 128:(q + 1) * 128],
                                         g[:, q * 128:(q + 1) * 128],
                                         w2blk_sb[:],
                                         start=True, stop=not has_dec_b2)
                    if has_dec_b2:
                        for half in range(2):
                            nc.tensor.matmul(ops[:, half * 512:(half + 1) * 512],
                                             ones_b[:], b2rep_sb[:],
                                             start=False, stop=True,
                                             skip_group_check=True)

                    x_sb = xsbp.tile([CH, MAXN * D], F32, tag="x_sb")
                    mrow = r * MAXN
                    mask_ap = (mask_sb[:, mrow:mrow + MAXN]
                               .unsqueeze(2).broadcast_to([CH, MAXN, D]))
                    nc.vector.tensor_tensor(
                        x_sb[:].rearrange("p (j d) -> p j d", d=D),
                        ops[:].rearrange("p (j d) -> p j d", d=D),
                        mask_ap, mybir.AluOpType.mult)

                    nc.sync.dma_start(out_x[r0 + r * CH:r0 + (r + 1) * CH, :],
                                      x_sb[:])
    nc.finalize()
    return nc


def kernel(z, sp_w1, sp_b1, sp_w2, sp_b2, pe_w1, pe_b1, pe_w2, pe_b2,
           dec_w1, dec_b1, dec_w2, dec_b2):
    global LAST_EXEC_NS, LAST_RESULTS
    z = np.ascontiguousarray(np.asarray(z, np.float32))
    sp_w1 = np.asarray(sp_w1, np.float32); sp_b1 = np.asarray(sp_b1, np.float32)
    sp_w2 = np.asarray(sp_w2, np.float32); sp_b2 = np.asarray(sp_b2, np.float32)
    pe_w1 = np.asarray(pe_w1, np.float32); pe_b1 = np.asarray(pe_b1, np.float32)
    pe_w2 = np.asarray(pe_w2, np.float32); pe_b2 = np.asarray(pe_b2, np.float32)
    dec_w1 = np.asarray(dec_w1, np.float32); dec_b1 = np.asarray(dec_b1, np.float32)
    dec_w2 = np.asarray(dec_w2, np.float32); dec_b2 = np.asarray(dec_b2, np.float32)

    # Host-side weight preprocessing (tiny, O(1) work): pe = MLP(I_16)
    # and position-modulated layer-1 weights.
    pe = _gelu_np(np.eye(MAXN, dtype=np.float32) @ pe_w1 + pe_b1) @ pe_w2 + pe_b2
    # W1all[i, p*HID + j] = pe[p, i] * dec_w1[i, j]
    w1all = (pe.T[:, :, None] * dec_w1[:, None, :]).reshape(H, MAXN * HID)
    w2blk = np.zeros((2 * HID, 2 * D), np.float32)
    w2blk[:HID, :D] = dec_w2
    w2blk[HID:, D:] = dec_w2
    b1pair = np.concatenate([dec_b1, dec_b1]).reshape(2 * HID, 1)
    b2rep = np.tile(dec_b2, 8).reshape(1, 8 * D)
    spw1h, spw1l = _split(sp_w1)
    spw2h, spw2l = _split(sp_w2)
    spb2h, spb2l = _split(sp_b2.reshape(1, MAXN))

    has_sp_b2 = bool(np.any(sp_b2))
    has_dec_b2 = bool(np.any(dec_b2))
    nc = build_nc(has_sp_b2, has_dec_b2)

    shared = {
        "w1all": _bf16(w1all),
        "w2blk": _bf16(w2blk),
        "b1pair": np.ascontiguousarray(b1pair, np.float32),
        "b2rep": _bf16(b2rep),
        "spw1h": spw1h, "spw1l": spw1l,
        "spb1": np.ascontiguousarray(sp_b1.reshape(SPH, 1)),
        "spw2h": spw2h, "spw2l": spw2l,
        "spb2h": spb2h, "spb2l": spb2l,
        "iotain": np.broadcast_to(np.arange(MAXN, dtype=np.int32)[None, :],
                                  (CH, MAXN)).copy(),
        "onesin": np.ones((1, CH), dtype=ml_dtypes.bfloat16),
    }
    in_maps = []
    for c in range(N_CORES):
        zT = np.ascontiguousarray(z[c * BSH:(c + 1) * BSH].T)  # [H, BSH] f32
        zh, zl = _split(zT)
        m = dict(shared)
        m["zTh"] = zh
        m["zTl"] = zl
        in_maps.append(m)

    trace = bool(int(os.environ.get("BASS_KERNEL_TRACE", "0")))
    if trace:
        results, LAST_EXEC_NS = _run_traced(nc, in_maps)
    else:
        res = run_bass_kernel_spmd(nc, in_maps, list(range(N_CORES)))
        results = res.results
        LAST_EXEC_NS = res.exec_time_ns
    LAST_RESULTS = results

    x = np.concatenate([np.asarray(results[c]["out_x"]).reshape(BSH, MAXN, D)
                        for c in range(N_CORES)], axis=0)
    n_pred = np.concatenate([np.asarray(results[c]["out_np"])
                             for c in range(N_CORES)], axis=0)
    mask = np.concatenate([np.asarray(results[c]["out_mask"])
                           for c in range(N_CORES)], axis=0) != 0
    batch = np.broadcast_to(np.arange(B, dtype=np.int32)[:, None],
                            (B, MAXN)).copy()
    return x, mask, batch, n_pred


def _run_traced(nc, in_maps):
    """Run once to warm (and get results), then re-run under the axon NTFF
    profile hook and parse the device-0 NTFF locally for exec time.

    The kernel must be compiled BEFORE the profile session opens —
    compiling inside an active capture wedges the execution (observed
    empirically), so the warm run also serves as the compile step.
    """
    import glob
    import tempfile

    from concourse import bass2jax

    import time
    _t0 = time.time()
    def _st(m):
        print(f"[trace {time.time()-_t0:6.1f}s] {m}", file=sys.stderr, flush=True)
    _st("warm run start")
    results = bass2jax.run_bass_via_pjrt(nc, in_maps, n_cores=N_CORES)
    _st("warm run done")

    exec_ns = None
    try:
        _ensure_axon_ntff_hook()
        from antenv.axon_hooks import get_axon_ntff_profile_hook
        hook = get_axon_ntff_profile_hook()
        if hook is None:
            print("no ntff hook; skipping timing", file=sys.stderr)
            return results, None
        prof_dir = tempfile.mkdtemp(prefix="ntff_")
        _st("entering profile hook")
        with hook(prof_dir, [0]):
            _st("profiled run start")
            bass2jax.run_bass_via_pjrt(nc, in_maps, n_cores=N_CORES)
            _st("profiled run done")
        _st("hook exited")
        ntffs = glob.glob(os.path.join(prof_dir, "*_body*.ntff"))
        if not ntffs:
            print(f"no _body ntffs in {prof_dir}: {os.listdir(prof_dir)}",
                  file=sys.stderr)
            return results, None
        import gauge.profiler as gp
        from concourse._compat import FishPath
        profile = gp.Profile(profile_path=FishPath(prof_dir),
                             kernel_dev_mode=True, profile_on_exit=False,
                             bass_kernel=nc.m, offline_processing=True,
                             fname="*_body*")
        _st("converting ntff to json")
        profile.convert_ntffs_to_json((0,))
        exec_ns = profile.get_total_time(0)
        _st(f"exec total_time = {exec_ns}")
        print(f"profile dir: {prof_dir}", file=sys.stderr)
        globals()["LAST_PROFILE_DIR"] = prof_dir
    except Exception as e:
        import traceback
        traceback.print_exc()
        print(f"tracing failed: {e}", file=sys.stderr)
    return results, exec_ns


def _ensure_axon_ntff_hook():
    """The agent image's antenv lacks axon_hooks; provide it and register
    the ctypes NTFF hook from trn_agent_boot so trace=True works."""
    try:
        from antenv.axon_hooks import get_axon_ntff_profile_hook  # noqa: F401
        return
    except ImportError:
        pass
    import types
    import antenv
    mod = types.ModuleType("antenv.axon_hooks")
    _state = {"hook": None}
    mod.set_axon_ntff_profile_hook = lambda h: _state.__setitem__("hook", h)
    mod.get_axon_ntff_profile_hook = lambda: _state["hook"]
    sys.modules["antenv.axon_hooks"] = mod
    antenv.axon_hooks = mod
    try:
        from trn_agent_boot.trn_boot import _ntff_profile_via_ctypes
        hook = _ntff_profile_via_ctypes("/opt/axon/libaxon_pjrt.so")
        mod.set_axon_ntff_profile_hook(hook)
    except Exception as e:  # degrade to no tracing
        print(f"ntff hook setup failed: {e}", file=sys.stderr)
